# revision 20
# baseline (speedup 1.0000x reference)
"""Trainium2 Bass kernel for the entity-assignment loss.

Math: per sample b, C[i,j] = mean_d (yt[b,i,d]-yp[b,j,d])^2.
loss = mean_b ( min_perm sum_i C[i, perm(i)] / 8 ).

Since each permutation uses every row i and every column j exactly once,
  sum_i C[i, perm(i)] = (nt + np - 2 * sum_i dot(i, perm(i))) / 64
with nt = sum_i |yt_i|^2, np = sum_j |yp_j|^2 (per-sample constants).
So min over perms only needs MAX over perms of the dot sum, computed with a
2^8 bitmask DP whose bit-i update is a perfectly strided access pattern.

Perf notes (measured on TRN2):
- every DVE instruction pays a ~70-130ns issue/SBUF-access overhead, and
  scalar_tensor_tensor runs at 1x (no DVE fast modes) -> the DP is
  overhead+exec bound at ~262ns per [128,128] op. 96 such ops is provably
  minimal for the bitmask DP (6 inner steps x 8 bits x 2 sample chunks;
  chunks cannot merge because the stt scalar is per-partition and two
  samples share each partition).
- GpSimd cannot help: TensorScalarPtr/TensorTensor are illegal opcodes on
  Pool in the TRN2 NEFF codegen (only Memset/DMA/custom-ISA kernels run
  there), so Pool only does the NEG memsets, overlapped with the loads.
- inputs are pre-cast to fp16 on the host: halves DMA bytes and removes the
  ScalarE cast + act-table load from the critical path; SQUARE norms on
  ScalarE overlap the DVE multiply.
- loads are quarter-granularity (tensor x chunk) and the product multiply is
  split per (chunk, i-half), so the first multiply starts ~1us before the
  last quarter lands.
- the fold tree (2x mode) replaces the segmented tensor_reduce (1x).

Sharding: pure data parallelism, 256 samples per core across 8 cores; the
final mean is taken on the host from per-sample partial results.
"""

import os
import sys

if "/opt/trn_rl_repo" not in sys.path:
    sys.path.insert(0, "/opt/trn_rl_repo")

import numpy as np

GPS = os.environ.get("K_GPS", "0") == "1"   # offload DP high bits to GpSimd
                                            # (dead on TRN2: TensorTensor /
                                            # TensorScalarPtr illegal on Pool)
N_GPS = int(os.environ.get("K_NGPS", "2"))  # how many of the 8 bits go to GpSimd

B, N, D = 2048, 8, 64
N_CORES = 8
B_LOC = B // N_CORES        # 256 samples per core
NT = 2                      # two samples per partition row (free-dim chunks)
NEG = -60000.0              # fp16-safe "minus infinity"

TRACE = False
_CACHE = {}


def _build():
    import concourse.bacc as bacc
    import concourse.mybir as mybir
    from concourse.tile import TileContext

    f32 = mybir.dt.float32
    f16 = mybir.dt.float16
    Alu = mybir.AluOpType
    Act = mybir.ActivationFunctionType

    nc = bacc.Bacc("TRN2", target_bir_lowering=False, debug=False)
    # row p holds samples 2p (first 512) and 2p+1 (next 512), fp16
    yt_d = nc.declare_dram_parameter("yt", [128, NT * N * D], f16, isOutput=False)
    yp_d = nc.declare_dram_parameter("yp", [128, NT * N * D], f16, isOutput=False)
    out_d = nc.declare_dram_parameter("out", [128, NT], f32, isOutput=True)

    gps_bits = set(range(N - N_GPS, N)) if GPS else set()

    with TileContext(nc) as tc:
        with (
            tc.tile_pool(name="io", bufs=1) as io_pool,
            tc.tile_pool(name="work", bufs=2) as work_pool,
            tc.tile_pool(name="res", bufs=1) as res_pool,
        ):
            loss_t = res_pool.tile([128, NT], f32, tag="loss")
            s_all = res_pool.tile([128, NT], f32, tag="s_all")
            G32 = res_pool.tile([128, NT * N * N], f32, tag="G32")
            dpa = res_pool.tile([128, NT * 256], f16, tag="dpa")
            dpb = res_pool.tile([128, NT * 256], f16, tag="dpb")
            cand = res_pool.tile([128, NT * N], f16, tag="cand")
            if GPS:
                accg = res_pool.tile([128, NT * 256], f16, tag="accg")
                candg = res_pool.tile([128, NT * 128], f16, tag="candg")

            # quarter-granularity loads: chunk-0 halves first so the first
            # multiply can start before chunk-1 data lands
            yt_t = io_pool.tile([128, NT * N * D], f16, tag="yt")
            yp_t = io_pool.tile([128, NT * N * D], f16, tag="yp")
            HW = N * D
            nc.sync.dma_start(out=yp_t[:, 0:HW], in_=yp_d[:, 0:HW])
            nc.sync.dma_start(out=yt_t[:, 0:HW], in_=yt_d[:, 0:HW])
            nc.sync.dma_start(out=yp_t[:, HW:2 * HW], in_=yp_d[:, HW:2 * HW])
            nc.sync.dma_start(out=yt_t[:, HW:2 * HW], in_=yt_d[:, HW:2 * HW])

            # DP state init on GpSimd, overlapped with the input DMA
            nc.gpsimd.memset(dpa[:, :], NEG)
            nc.gpsimd.memset(dpb[:, :], NEG)
            if GPS:
                nc.gpsimd.memset(accg[:, :], NEG)

            # G matrices, both chunks in each op (halves the per-instruction
            # SBUF-access bubbles): broadcast multiply, three binary folds
            # over d, then a segmented reduce; norms on ScalarE in parallel
            nt_h = [None] * NT
            np_h = [None] * NT
            for h in range(NT):
                sq = work_pool.tile([128, N * D], f32, tag="sq")
                nt_h[h] = work_pool.tile([128, 1], f32, tag="nt", name=f"nt{h}")
                np_h[h] = work_pool.tile([128, 1], f32, tag="npt", name=f"npt{h}")
                nc.scalar.activation(out=sq[:, :],
                                     in_=yt_t[:, h * N * D:(h + 1) * N * D],
                                     func=Act.Square, accum_out=nt_h[h][:, 0:1])
                nc.scalar.activation(out=sq[:, :],
                                     in_=yp_t[:, h * N * D:(h + 1) * N * D],
                                     func=Act.Square, accum_out=np_h[h][:, 0:1])

            # multiplies split per (chunk, i-half) so the first starts as
            # soon as the first two quarter-loads land
            prod = work_pool.tile([128, NT * N * N * D], f16, tag="prod")
            for h in range(NT):
                for ih in range(2):
                    yt_b = yt_t[:, h * HW + ih * HW // 2:
                                h * HW + (ih + 1) * HW // 2] \
                        .rearrange("p (i d) -> p i d", d=D).unsqueeze(2) \
                        .broadcast_to([128, N // 2, N, D])
                    yp_b = yp_t[:, h * HW:(h + 1) * HW] \
                        .rearrange("p (j d) -> p j d", d=D).unsqueeze(1) \
                        .broadcast_to([128, N // 2, N, D])
                    nc.vector.tensor_tensor(
                        out=prod[:, (2 * h + ih) * N * N * D // 2:
                                 (2 * h + ih + 1) * N * N * D // 2]
                            .rearrange("p (i j d) -> p i j d", j=N, d=D),
                        in0=yt_b, in1=yp_b, op=Alu.mult)
            pv = prod.rearrange("p (q d) -> p q d", d=D)
            half = work_pool.tile([128, NT * N * N * D // 2], f16, tag="half")
            hv = half.rearrange("p (q d) -> p q d", d=D // 2)
            nc.vector.tensor_tensor(
                out=hv, in0=pv[:, :, 0:D // 2], in1=pv[:, :, D // 2:D],
                op=Alu.add)
            quart = work_pool.tile([128, NT * N * N * D // 4], f16, tag="quart")
            qv = quart.rearrange("p (q d) -> p q d", d=D // 4)
            nc.vector.tensor_tensor(
                out=qv, in0=hv[:, :, 0:D // 4], in1=hv[:, :, D // 4:D // 2],
                op=Alu.add)
            eighth = work_pool.tile([128, NT * N * N * D // 8], f16, tag="eighth")
            ev = eighth.rearrange("p (q d) -> p q d", d=D // 8)
            nc.vector.tensor_tensor(
                out=ev, in0=qv[:, :, 0:D // 8], in1=qv[:, :, D // 8:D // 4],
                op=Alu.add)
            # finish with three more folds instead of a segmented
            # tensor_reduce: the folds run in DVE 2x mode, the reduce is 1x
            s16 = work_pool.tile([128, NT * N * N * D // 16], f16, tag="s16")
            sv = s16.rearrange("p (q d) -> p q d", d=D // 16)
            nc.vector.tensor_tensor(
                out=sv, in0=ev[:, :, 0:D // 16], in1=ev[:, :, D // 16:D // 8],
                op=Alu.add)
            s32 = work_pool.tile([128, NT * N * N * D // 32], f16, tag="s32")
            wv = s32.rearrange("p (q d) -> p q d", d=D // 32)
            nc.vector.tensor_tensor(
                out=wv, in0=sv[:, :, 0:D // 32], in1=sv[:, :, D // 32:D // 16],
                op=Alu.add)
            nc.vector.tensor_tensor(
                out=G32.rearrange("p (q e) -> p q e", e=1),
                in0=wv[:, :, 0:1], in1=wv[:, :, 1:2], op=Alu.add)

            for h in range(NT):
                nc.vector.tensor_add(s_all[:, h:h + 1], nt_h[h][:, 0:1],
                                     np_h[h][:, 0:1])

            # bitmask DP over both chunks: states laid out [chunk, state]
            g_v = G32.rearrange("p (h q) -> p h q", h=NT)
            bufs = [dpa, dpb]
            for k in range(N):
                old = bufs[k % 2]
                new = bufs[(k + 1) % 2]
                if k == 0:
                    # singletons, pairwise-merged: targets {2^i, 2^(i+1)}
                    # are stride-2^i; G cols {i*8, (i+1)*8} are stride-8.
                    for i in range(0, N, 2):
                        ci = 2 ** i
                        nv = new.rearrange("p (h s) -> p h s", h=NT)
                        tgt = nv[:, :, ci:2 * ci + 1:ci]
                        gsrc = g_v[:, :, i * N:(i + 2) * N:N]
                        nc.vector.tensor_copy(tgt, gsrc)
                    continue
                if k == N - 1:
                    # final column: collect the 8 candidates densely; cand
                    # slot order ascends with source state (reduce_max is
                    # order-invariant).
                    for i in range(0, N, 2):
                        ci = 2 ** i
                        ov = old.rearrange("p (h s) -> p h s", h=NT)
                        src = ov[:, :, 255 - 2 * ci:256 - ci:ci]
                        cv = cand.rearrange("p (h s) -> p h s", h=NT)[:, :, i:i + 2]
                        gsrc = g_v[:, :, (i + 1) * N + k::-N][:, :, 0:2]
                        nc.vector.tensor_tensor(out=cv, in0=src, in1=gsrc,
                                                op=Alu.add)
                    continue
                # inner steps: DVE handles low bits in-place on `new`;
                # GpSimd handles high bits into its own accumulator `accg`
                # (TensorScalarPtr is illegal on Pool, so it uses a
                # tensor_tensor add with a broadcast G column + a max),
                # max-merged into `new` once per chunk at end of step.
                for i in range(N):
                    ci = 2 ** i
                    col = i * N + k
                    a = 256 // (2 * ci)
                    vo = old.rearrange("p (h a b c) -> p h a b c",
                                       h=NT, b=2, c=ci)
                    src = vo[:, :, :, 0, :]
                    if i in gps_bits:
                        vn = accg.rearrange("p (h a b c) -> p h a b c",
                                            h=NT, b=2, c=ci)
                        tgt = vn[:, :, :, 1, :]
                        gb = g_v[:, :, col:col + 1].unsqueeze(3) \
                            .broadcast_to([128, NT, a, ci])
                        cg = candg.rearrange("p (h a c) -> p h a c",
                                             h=NT, c=ci)
                        nc.gpsimd.tensor_tensor(out=cg, in0=src, in1=gb,
                                                op=Alu.add)
                        nc.gpsimd.tensor_tensor(out=tgt, in0=tgt, in1=cg,
                                                op=Alu.max)
                        continue
                    vn = new.rearrange("p (h a b c) -> p h a b c",
                                       h=NT, b=2, c=ci)
                    tgt = vn[:, :, :, 1, :]
                    for h in range(NT):
                        nc.vector.scalar_tensor_tensor(
                            out=tgt[:, h], in0=src[:, h],
                            scalar=G32[:, h * N * N + col:h * N * N + col + 1],
                            in1=tgt[:, h], op0=Alu.add, op1=Alu.max)
                if gps_bits:
                    nc.vector.tensor_tensor(
                        out=new[:, :], in0=new[:, :], in1=accg[:, :],
                        op=Alu.max)

            dmax = res_pool.tile([128, NT], f16, tag="dmax")
            nc.vector.tensor_reduce(
                out=dmax[:, :],
                in_=cand.rearrange("p (h s) -> p h s", h=NT),
                axis=mybir.AxisListType.X, op=Alu.max)
            nc.vector.scalar_tensor_tensor(
                out=loss_t[:, :],
                in0=dmax[:, :],
                scalar=-2.0,
                in1=s_all[:, :],
                op0=Alu.mult,
                op1=Alu.add,
            )
            nc.sync.dma_start(out=out_d[:, :], in_=loss_t[:, :])
    nc.compile()
    return nc


def kernel(y_true: np.ndarray, y_pred: np.ndarray) -> np.ndarray:
    from concourse.bass_utils import run_bass_kernel_spmd

    if "nc" not in _CACHE:
        _CACHE["nc"] = _build()
    nc = _CACHE["nc"]

    yt = np.ascontiguousarray(
        np.asarray(y_true, dtype=np.float16).reshape(B, N * D))
    yp = np.ascontiguousarray(
        np.asarray(y_pred, dtype=np.float16).reshape(B, N * D))

    in_maps = [
        {
            "yt": np.ascontiguousarray(
                yt[c * B_LOC:(c + 1) * B_LOC].reshape(128, NT * N * D)),
            "yp": np.ascontiguousarray(
                yp[c * B_LOC:(c + 1) * B_LOC].reshape(128, NT * N * D)),
        }
        for c in range(N_CORES)
    ]
    res = run_bass_kernel_spmd(nc, in_maps, list(range(N_CORES)), trace=TRACE)
    _CACHE["last_results"] = res
    vals = np.concatenate([np.asarray(r["out"], dtype=np.float64).reshape(-1)
                           for r in res.results])
    loss = vals.mean() / (D * N)
    return np.float32(loss)


# revision 23
# speedup vs baseline: 1.1026x; 1.1026x over previous
"""Trainium2 Bass kernel for the entity-assignment loss.

Math: per sample b, C[i,j] = mean_d (yt[b,i,d]-yp[b,j,d])^2.
loss = mean_b ( min_perm sum_i C[i, perm(i)] / 8 ).

Since each permutation uses every row i and every column j exactly once,
  sum_i C[i, perm(i)] = (nt + np - 2 * sum_i dot(i, perm(i))) / 64
with nt = sum_i |yt_i|^2, np = sum_j |yp_j|^2 (per-sample constants).
So min over perms only needs MAX over perms of the dot sum, computed with a
2^8 bitmask DP whose bit-i update is a perfectly strided access pattern.

Perf notes (measured on TRN2):
- every DVE instruction pays a ~70-130ns issue/SBUF-access overhead, and
  scalar_tensor_tensor runs at 1x (no DVE fast modes) -> the DP is
  overhead+exec bound at ~262ns per [128,128] op. 96 such ops is provably
  minimal for the bitmask DP (6 inner steps x 8 bits x 2 sample chunks;
  chunks cannot merge because the stt scalar is per-partition and two
  samples share each partition).
- GpSimd cannot help: TensorScalarPtr/TensorTensor are illegal opcodes on
  Pool in the TRN2 NEFF codegen (only Memset/DMA/custom-ISA kernels run
  there), so Pool only does the NEG memsets, overlapped with the loads.
- inputs are pre-cast to fp16 on the host: halves DMA bytes and removes the
  ScalarE cast + act-table load from the critical path; SQUARE norms on
  ScalarE overlap the DVE multiply.
- loads are quarter-granularity (tensor x chunk) and the product multiply is
  split per (chunk, i-half), so the first multiply starts ~1us before the
  last quarter lands.
- the fold tree (2x mode) replaces the segmented tensor_reduce (1x).

Sharding: pure data parallelism, 256 samples per core across 8 cores; the
final mean is taken on the host from per-sample partial results.
"""

import os
import sys

if "/opt/trn_rl_repo" not in sys.path:
    sys.path.insert(0, "/opt/trn_rl_repo")

import numpy as np

B, N, D = 2048, 8, 64
N_CORES = 8
B_LOC = B // N_CORES        # 256 samples per core
NT = 2                      # two samples per partition row (free-dim chunks)
NEG = -60000.0              # fp16-safe "minus infinity"

TRACE = False
_CACHE = {}


def _build():
    import concourse.bacc as bacc
    import concourse.mybir as mybir
    from concourse.tile import TileContext

    f32 = mybir.dt.float32
    f16 = mybir.dt.float16
    Alu = mybir.AluOpType
    Act = mybir.ActivationFunctionType

    nc = bacc.Bacc("TRN2", target_bir_lowering=False, debug=False)
    # row p holds samples 2p (first 512) and 2p+1 (next 512), fp16
    yt_d = nc.declare_dram_parameter("yt", [128, NT * N * D], f16, isOutput=False)
    yp_d = nc.declare_dram_parameter("yp", [128, NT * N * D], f16, isOutput=False)
    out_d = nc.declare_dram_parameter("out", [128, NT], f32, isOutput=True)

    with TileContext(nc) as tc:
        with (
            tc.tile_pool(name="io", bufs=1) as io_pool,
            tc.tile_pool(name="work", bufs=2) as work_pool,
            tc.tile_pool(name="res", bufs=1) as res_pool,
        ):
            loss_t = res_pool.tile([128, NT], f32, tag="loss")
            s_all = res_pool.tile([128, NT], f32, tag="s_all")
            G32 = res_pool.tile([128, NT * N * N], f32, tag="G32")
            dpa = res_pool.tile([128, NT * 256], f16, tag="dpa")
            dpb = res_pool.tile([128, NT * 256], f16, tag="dpb")
            cand = res_pool.tile([128, NT * N], f16, tag="cand")

            # quarter-granularity loads: chunk-0 halves first so the first
            # multiply can start before chunk-1 data lands
            yt_t = io_pool.tile([128, NT * N * D], f16, tag="yt")
            yp_t = io_pool.tile([128, NT * N * D], f16, tag="yp")
            HW = N * D
            nc.sync.dma_start(out=yp_t[:, 0:HW], in_=yp_d[:, 0:HW])
            nc.sync.dma_start(out=yt_t[:, 0:HW], in_=yt_d[:, 0:HW])
            nc.sync.dma_start(out=yp_t[:, HW:2 * HW], in_=yp_d[:, HW:2 * HW])
            nc.sync.dma_start(out=yt_t[:, HW:2 * HW], in_=yt_d[:, HW:2 * HW])

            # DP state init on GpSimd, overlapped with the input DMA
            nc.gpsimd.memset(dpa[:, :], NEG)
            nc.gpsimd.memset(dpb[:, :], NEG)

            # G matrices, both chunks in each op (halves the per-instruction
            # SBUF-access bubbles): broadcast multiply, three binary folds
            # over d, then a segmented reduce; norms on ScalarE in parallel
            nt_h = [None] * NT
            np_h = [None] * NT
            for h in range(NT):
                sq = work_pool.tile([128, N * D], f32, tag="sq")
                nt_h[h] = work_pool.tile([128, 1], f32, tag="nt", name=f"nt{h}")
                np_h[h] = work_pool.tile([128, 1], f32, tag="npt", name=f"npt{h}")
                nc.scalar.activation(out=sq[:, :],
                                     in_=yt_t[:, h * N * D:(h + 1) * N * D],
                                     func=Act.Square, accum_out=nt_h[h][:, 0:1])
                nc.scalar.activation(out=sq[:, :],
                                     in_=yp_t[:, h * N * D:(h + 1) * N * D],
                                     func=Act.Square, accum_out=np_h[h][:, 0:1])

            # multiplies split per (chunk, i-half) so the first starts as
            # soon as the first two quarter-loads land
            prod = work_pool.tile([128, NT * N * N * D], f16, tag="prod")
            for h in range(NT):
                for ih in range(2):
                    yt_b = yt_t[:, h * HW + ih * HW // 2:
                                h * HW + (ih + 1) * HW // 2] \
                        .rearrange("p (i d) -> p i d", d=D).unsqueeze(2) \
                        .broadcast_to([128, N // 2, N, D])
                    yp_b = yp_t[:, h * HW:(h + 1) * HW] \
                        .rearrange("p (j d) -> p j d", d=D).unsqueeze(1) \
                        .broadcast_to([128, N // 2, N, D])
                    nc.vector.tensor_tensor(
                        out=prod[:, (2 * h + ih) * N * N * D // 2:
                                 (2 * h + ih + 1) * N * N * D // 2]
                            .rearrange("p (i j d) -> p i j d", j=N, d=D),
                        in0=yt_b, in1=yp_b, op=Alu.mult)
            pv = prod.rearrange("p (q d) -> p q d", d=D)
            half = work_pool.tile([128, NT * N * N * D // 2], f16, tag="half")
            hv = half.rearrange("p (q d) -> p q d", d=D // 2)
            nc.vector.tensor_tensor(
                out=hv, in0=pv[:, :, 0:D // 2], in1=pv[:, :, D // 2:D],
                op=Alu.add)
            quart = work_pool.tile([128, NT * N * N * D // 4], f16, tag="quart")
            qv = quart.rearrange("p (q d) -> p q d", d=D // 4)
            nc.vector.tensor_tensor(
                out=qv, in0=hv[:, :, 0:D // 4], in1=hv[:, :, D // 4:D // 2],
                op=Alu.add)
            eighth = work_pool.tile([128, NT * N * N * D // 8], f16, tag="eighth")
            ev = eighth.rearrange("p (q d) -> p q d", d=D // 8)
            nc.vector.tensor_tensor(
                out=ev, in0=qv[:, :, 0:D // 8], in1=qv[:, :, D // 8:D // 4],
                op=Alu.add)
            # finish with three more folds instead of a segmented
            # tensor_reduce: the folds run in DVE 2x mode, the reduce is 1x
            s16 = work_pool.tile([128, NT * N * N * D // 16], f16, tag="s16")
            sv = s16.rearrange("p (q d) -> p q d", d=D // 16)
            nc.vector.tensor_tensor(
                out=sv, in0=ev[:, :, 0:D // 16], in1=ev[:, :, D // 16:D // 8],
                op=Alu.add)
            s32 = work_pool.tile([128, NT * N * N * D // 32], f16, tag="s32")
            wv = s32.rearrange("p (q d) -> p q d", d=D // 32)
            nc.vector.tensor_tensor(
                out=wv, in0=sv[:, :, 0:D // 32], in1=sv[:, :, D // 32:D // 16],
                op=Alu.add)
            nc.vector.tensor_tensor(
                out=G32.rearrange("p (q e) -> p q e", e=1),
                in0=wv[:, :, 0:1], in1=wv[:, :, 1:2], op=Alu.add)

            for h in range(NT):
                nc.vector.tensor_add(s_all[:, h:h + 1], nt_h[h][:, 0:1],
                                     np_h[h][:, 0:1])

            # bitmask DP over both chunks: states laid out [chunk, state]
            g_v = G32.rearrange("p (h q) -> p h q", h=NT)
            bufs = [dpa, dpb]
            for k in range(N):
                old = bufs[k % 2]
                new = bufs[(k + 1) % 2]
                if k == 0:
                    # singletons, pairwise-merged: targets {2^i, 2^(i+1)}
                    # are stride-2^i; G cols {i*8, (i+1)*8} are stride-8.
                    for i in range(0, N, 2):
                        ci = 2 ** i
                        nv = new.rearrange("p (h s) -> p h s", h=NT)
                        tgt = nv[:, :, ci:2 * ci + 1:ci]
                        gsrc = g_v[:, :, i * N:(i + 2) * N:N]
                        nc.vector.tensor_copy(tgt, gsrc)
                    continue
                if k == N - 1:
                    # final column: collect the 8 candidates densely; cand
                    # slot order ascends with source state (reduce_max is
                    # order-invariant).
                    for i in range(0, N, 2):
                        ci = 2 ** i
                        ov = old.rearrange("p (h s) -> p h s", h=NT)
                        src = ov[:, :, 255 - 2 * ci:256 - ci:ci]
                        cv = cand.rearrange("p (h s) -> p h s", h=NT)[:, :, i:i + 2]
                        gsrc = g_v[:, :, (i + 1) * N + k::-N][:, :, 0:2]
                        nc.vector.tensor_tensor(out=cv, in0=src, in1=gsrc,
                                                op=Alu.add)
                    continue
                # inner steps, all on DVE (no other engine can run
                # TensorScalarPtr/TensorTensor on TRN2)
                for i in range(N):
                    ci = 2 ** i
                    col = i * N + k
                    a = 256 // (2 * ci)
                    # popcount range trim: at step k only targets with
                    # popcount k+1 matter. With state = hi*(2ci) + bit_i*ci
                    # + lo, restrict hi/lo to the value range that covers
                    # every split of the k other bits across the 7-i high
                    # and i low positions. Unwritten slots keep NEG/stale
                    # values, which stay valid lower bounds.
                    km_hi = min(k, N - 1 - i)
                    hi0 = 2 ** max(0, k - i) - 1
                    hi1 = 2 ** (N - 1 - i) - 2 ** (N - 1 - i - km_hi)
                    lo0 = 2 ** max(0, k - (N - 1 - i)) - 1
                    lo1 = (2 ** i - 2 ** (i - min(k, i))) if i > 0 else 0
                    vo = old.rearrange("p (h a b c) -> p h a b c",
                                       h=NT, b=2, c=ci)
                    src = vo[:, :, hi0:hi1 + 1, 0, lo0:lo1 + 1]
                    vn = new.rearrange("p (h a b c) -> p h a b c",
                                       h=NT, b=2, c=ci)
                    tgt = vn[:, :, hi0:hi1 + 1, 1, lo0:lo1 + 1]
                    for h in range(NT):
                        nc.vector.scalar_tensor_tensor(
                            out=tgt[:, h], in0=src[:, h],
                            scalar=G32[:, h * N * N + col:h * N * N + col + 1],
                            in1=tgt[:, h], op0=Alu.add, op1=Alu.max)

            dmax = res_pool.tile([128, NT], f16, tag="dmax")
            nc.vector.tensor_reduce(
                out=dmax[:, :],
                in_=cand.rearrange("p (h s) -> p h s", h=NT),
                axis=mybir.AxisListType.X, op=Alu.max)
            nc.vector.scalar_tensor_tensor(
                out=loss_t[:, :],
                in0=dmax[:, :],
                scalar=-2.0,
                in1=s_all[:, :],
                op0=Alu.mult,
                op1=Alu.add,
            )
            nc.sync.dma_start(out=out_d[:, :], in_=loss_t[:, :])
    nc.compile()
    return nc


def kernel(y_true: np.ndarray, y_pred: np.ndarray) -> np.ndarray:
    from concourse.bass_utils import run_bass_kernel_spmd

    if "nc" not in _CACHE:
        _CACHE["nc"] = _build()
    nc = _CACHE["nc"]

    yt = np.ascontiguousarray(
        np.asarray(y_true, dtype=np.float16).reshape(B, N * D))
    yp = np.ascontiguousarray(
        np.asarray(y_pred, dtype=np.float16).reshape(B, N * D))

    in_maps = [
        {
            "yt": np.ascontiguousarray(
                yt[c * B_LOC:(c + 1) * B_LOC].reshape(128, NT * N * D)),
            "yp": np.ascontiguousarray(
                yp[c * B_LOC:(c + 1) * B_LOC].reshape(128, NT * N * D)),
        }
        for c in range(N_CORES)
    ]
    res = run_bass_kernel_spmd(nc, in_maps, list(range(N_CORES)), trace=TRACE)
    _CACHE["last_results"] = res
    vals = np.concatenate([np.asarray(r["out"], dtype=np.float64).reshape(-1)
                           for r in res.results])
    loss = vals.mean() / (D * N)
    return np.float32(loss)
